# revision 46
# baseline (speedup 1.0000x reference)
"""Trainium2 Bass kernel for nn_CrossAttention (dense_transformer).

Sharding: data-parallel over batch B=8 across 8 NeuronCores (1 sample
per core). BatchNorm uses batch statistics, so per-channel partial
[sum, sumsq] are all-reduced across cores ([128,2] f32 payload, 2x).

Per-core layout: activations [C=128 partitions, N=H*W=2304 free], all
bf16 (HBM traffic halved vs f32; bf16 matmuls stream 1 row/cycle).

Attention in energy-transposed layout with algebraically folded
projections:
  attn1: energy^T[m,q] = sum_c yp[c,m] * qh1[c,q], qh1 = (Wq^T Wk)^T xp
  attn2: energy^T[m,q] = sum_c ky2[c,m] * h2[c,q], ky2 = (Wq^T Wk) yp
(the attn2 form moves the projection matmul onto the y side so it can
be built inside the BN-1 allreduce bubble, and h2 feeds the energy
matmul directly as the moving operand).

Softmax over m (= partitions) without max-subtraction, rescaled by
exp(-3) via the ACT free bias so exp output fits fp8e5 (|E| <= ~12 ->
ee <= e^9; the constant cancels between numerator and denominator).
ee is written as fp8e5 directly by the EXP activation. The value path
runs in fp8 DoubleRow mode (2 rows/cycle, contraction 256 = one
m-chunk pair per matmul): one matmul per pair for attn-out (lhsT =
vT[2j:2j+2] fp8e4, gamma folded in) and one accumulating ones-matmul
per pair for the softmax denominator -- the denominator lands in PSUM
(f32-exact, pre-broadcast across partitions), replacing the old
DVE/GpSimd pair-sum + fold-tree entirely.

PSUM (8 banks, bank-rounded): pE 2x[128,2,512] (4 banks; also hosts
proj/vT/qh/ky side-work scratch via shared tag), pAcc 2x[128,512]
(attn-out accum / conv taps / pred head), pCS 2x[128,512] (softmax
denominator accum).

conv3x3 = 9 shifted-window bf16 matmuls over a zero-padded
[128,50,50] buffer. BN batch stats go through the ncfw AllReduce;
vT2+ky2 are built inside the first allreduce window.

Input DMA: weights needed by the prelude are emitted first, then
x/y chunk pieces in consumption order (descriptor generation is
serial at ~0.6us each, so emission order is queue-start order), then
late-needed conv weights.
"""

import sys

sys.path.insert(0, "/opt/trn_rl_repo")

import numpy as np

_NC_CACHE = {}

B, CIN, C, H, W = 8, 256, 128, 48, 48
N = H * W  # 2304
P = 128
NKO = CIN // P  # 2
NMO = N // P  # 18
NPAIR = NMO // 2  # 9
# q superblocks: row-aligned chunks (48-col rows); 480 = 10 rows
QCH = [(0, 480), (480, 480), (960, 480), (1440, 480), (1920, 384)]
ROWCH = [(0, 10), (10, 10), (20, 10), (30, 10), (40, 8)]
# vT[mo] becomes computable once yp chunks covering cols [128mo,128mo+128)
# are projected; chunk c covers cols [480c, 480c+480)
VT_BY_CHUNK = [[0, 1, 2], [3, 4, 5, 6], [7, 8, 9, 10], [11, 12, 13, 14], [15, 16, 17]]
NSTAT = float(B * N)  # BN stat count over (B,H,W)
EPS = 1e-5
EXP_BIAS = -3.0  # exp(E-3): cancels in softmax, keeps ee in fp8e5 range


def _build(variant="default"):
    """variant: 'default' = 8-core w/ collectives; 'sim' = single-core,
    collectives replaced by DMA copy (for TimelineSim profiling)."""
    key = f"nc_{variant}"
    if key in _NC_CACHE:
        return _NC_CACHE[key]

    import concourse.mybir as mybir
    import concourse.tile as tile
    from concourse import bacc
    from contextlib import ExitStack

    F32 = mybir.dt.float32
    BF16 = mybir.dt.bfloat16
    FP8E4 = mybir.dt.float8e4
    FP8E5 = mybir.dt.float8e5
    AF = mybir.ActivationFunctionType
    ALU = mybir.AluOpType
    AX = mybir.AxisListType
    DR = mybir.MatmulPerfMode.DoubleRow

    sim = variant == "sim"
    nc = bacc.Bacc(
        "TRN2", target_bir_lowering=False, debug=False,
        num_devices=1 if sim else 8,
    )

    # ---- DRAM I/O ----
    d_x = nc.dram_tensor("x", [CIN, N], BF16, kind="ExternalInput")
    d_y = nc.dram_tensor("y", [CIN, N], BF16, kind="ExternalInput")
    d_w_inT = nc.dram_tensor("w_inT", [P, NKO, P], BF16, kind="ExternalInput")
    d_b_in = nc.dram_tensor("b_in", [P, 1], F32, kind="ExternalInput")
    d_A1 = nc.dram_tensor("A1", [P, P], BF16, kind="ExternalInput")
    d_gwv1T = nc.dram_tensor("gwv1T", [P, P], BF16, kind="ExternalInput")
    d_A2T = nc.dram_tensor("A2T", [P, P], BF16, kind="ExternalInput")
    d_gwv2T = nc.dram_tensor("gwv2T", [P, P], BF16, kind="ExternalInput")
    d_w1T = nc.dram_tensor("w1T", [P, 9, P], BF16, kind="ExternalInput")
    d_bn1s = nc.dram_tensor("bn1s", [P, 1], F32, kind="ExternalInput")
    d_bn1b = nc.dram_tensor("bn1b", [P, 1], F32, kind="ExternalInput")
    d_w2T = nc.dram_tensor("w2T", [P, 9, P], BF16, kind="ExternalInput")
    d_bn2s = nc.dram_tensor("bn2s", [P, 1], F32, kind="ExternalInput")
    d_bn2b = nc.dram_tensor("bn2b", [P, 1], F32, kind="ExternalInput")
    d_predT = nc.dram_tensor("predT", [P, P], BF16, kind="ExternalInput")
    d_pred_b = nc.dram_tensor("pred_b", [1, 1], F32, kind="ExternalInput")
    d_out = nc.dram_tensor("out", [1, N], F32, kind="ExternalOutput")

    with tile.TileContext(nc) as tc, ExitStack() as ctx:
        wgt = ctx.enter_context(tc.tile_pool(name="wgt", bufs=1))
        act = ctx.enter_context(tc.tile_pool(name="act", bufs=1))
        ew = ctx.enter_context(tc.tile_pool(name="ew", bufs=1))
        eeP = ctx.enter_context(tc.tile_pool(name="eeP", bufs=4))
        load = ctx.enter_context(tc.tile_pool(name="load", bufs=4))
        dram = ctx.enter_context(tc.tile_pool(name="dram", bufs=1, space="DRAM"))
        # PSUM, exactly 8 banks:
        #   pE   2x[128,2,512] f32 (4 banks) energy pairs + side-work scratch
        #   pAcc 2x[128,512]   f32 (2 banks) attn-out accum / conv / pred
        #   pCS  2x[128,512]   f32 (2 banks) softmax denominator accum
        pE = ctx.enter_context(tc.tile_pool(name="pE", bufs=2, space="PSUM"))
        pAcc = ctx.enter_context(tc.tile_pool(name="pAcc", bufs=2, space="PSUM"))
        pCS = ctx.enter_context(tc.tile_pool(name="pCS", bufs=2, space="PSUM"))

        # ---------- input DMA: prelude weights first, then x/y chunks in
        # consumption order, then late-needed conv weights. Descriptor
        # generation is serial (~0.6us each) so emission order = start order.
        xr = [load.tile([P, N], BF16, tag="in_r", name=f"xr{k}") for k in range(NKO)]
        yr = [load.tile([P, N], BF16, tag="in_r", name=f"yr{k}") for k in range(NKO)]
        xp = act.tile([P, N], BF16, tag="tagA")
        yp = act.tile([P, N], BF16, tag="tagB")

        def emit_load(dsrc, rr, c):
            q0, qn = QCH[c]
            for ko in range(NKO):
                nc.sync.dma_start(
                    rr[ko][:, q0 : q0 + qn],
                    dsrc[ko * P : (ko + 1) * P, q0 : q0 + qn],
                )

        def load_w(dsrc, shape, tag, dtype=BF16):
            t = wgt.tile(shape, dtype, tag=tag)
            nc.sync.dma_start(t[:], dsrc[...])
            return t

        # prelude weights (small, needed first); A1/gwv1T are consumed a
        # couple of microseconds into the prelude, so their descriptors are
        # generated after the chunk-0 input pieces (desc-gen is serial and
        # gates the first projection matmul)
        w_inT_r = load_w(d_w_inT, [P, NKO, P], "w_inT_r")
        b_in = load_w(d_b_in, [P, 1], "b_in", F32)
        emit_load(d_y, yr, 0)
        gwv1T_r = load_w(d_gwv1T, [P, P], "gwv1T_r")
        emit_load(d_x, xr, 0)
        A1_r = load_w(d_A1, [P, P], "A1_r")
        for c in range(1, 5):
            emit_load(d_y, yr, c)
        for c in range(1, 5):
            emit_load(d_x, xr, c)
        # late-needed weights after all input pieces
        A2T_r = load_w(d_A2T, [P, P], "A2T_r")
        gwv2T_r = load_w(d_gwv2T, [P, P], "gwv2T_r")
        w1T_r = load_w(d_w1T, [P, 9, P], "w1T_r")
        w2T_r = load_w(d_w2T, [P, 9, P], "w2T_r")
        predT_r = load_w(d_predT, [P, P], "predT_r")
        bn1s = load_w(d_bn1s, [P, 1], "bn1s", F32)
        bn1b = load_w(d_bn1b, [P, 1], "bn1b", F32)
        bn2s = load_w(d_bn2s, [P, 1], "bn2s", F32)
        bn2b = load_w(d_bn2b, [P, 1], "bn2b", F32)
        pred_b = load_w(d_pred_b, [1, 1], "pred_b", F32)

        # all-ones fp8e4 stationary for the DoubleRow denominator matmul
        ones_f = wgt.tile([P, 2 * P], F32, tag="ones_f")
        nc.gpsimd.memset(ones_f[:], 1.0)
        ones8 = wgt.tile([P, 2, P], FP8E4, tag="ones8")
        nc.vector.tensor_copy(ones8[:], ones_f[:].rearrange("p (t q) -> p t q", q=P))

        # warm the PE clock-gate during the input-DMA wait: these matmuls
        # need no DMA data (memset source) and finish before the first real
        # matmul's inputs arrive (~13us), so the prelude starts at 2.4GHz
        # instead of paying the ~3.4us HAM ramp
        warm_src = wgt.tile([P, 512], BF16, tag="warm_src")
        nc.gpsimd.memset(warm_src[:], 1.0)
        for i in range(20):
            ps = pE.tile([P, 2, 512], F32, tag="energy", name="warm_ps")
            nc.tensor.matmul(
                ps[0:1, 0, :512], warm_src[:, 0:1], warm_src[:],
                start=True, stop=True,
            )

        zrow = wgt.tile([P, W + 2], BF16, tag="zrow")
        nc.gpsimd.memset(zrow[:], 0.0)

        exp_bias = wgt.tile([P, 1], F32, tag="exp_bias")
        nc.gpsimd.memset(exp_bias[:], EXP_BIAS)

        # BN-stats exchange via XOR-relative remote SBUF DMA (peer k of core
        # i is i^k): each core broadcasts its [P,8] stats row to all 7 peers
        # (each send bumps the receiver's round sem by 2 -> wait 14), then
        # reduces the gathered slots locally. This replaces the ncfw mesh
        # AllReduce (~8-10us of post-arrival software per call) with ~1-2us
        # of direct DMA. The receive wait depends on remote increments the
        # tile scheduler's single-core sim cannot model, so those wait
        # values are zeroed during scheduling and restored before codegen
        # (see the swap below TileContext exit).
        # Measured A/B on this environment: the ncfw mesh AllReduce beats the
        # remote-SBUF-DMA exchange by ~50us (the rdma deliveries + prelude
        # AllGather are slow here), so rdma stays off by default.
        import os as _os
        rdma = (not sim) and _os.environ.get("KERNEL_RDMA", "0") == "1"
        rdma_waits = []
        if rdma:
            rs1 = nc.alloc_semaphore("bn_rsem1")
            rs2 = nc.alloc_semaphore("bn_rsem2")
            lsem = nc.alloc_semaphore("bn_lsem")
            nc.gpsimd.sem_clear(rs1)
            nc.gpsimd.sem_clear(rs2)
            nc.gpsimd.sem_clear(lsem)
        gths = [
            wgt.tile([P, 8, 8], F32, tag="gth", name=f"gth_{r}") for r in (0, 1)
        ]

        def emit_stat_preps(r):
            # emitted right after the stats writes: the prep's deferred
            # source-read then RAW-orders the whole group after the stats
            for k in range(1, 8):
                rd = [None] * 8
                rd[k] = (0, k)
                bp = nc.gpsimd.remote_dma_broadcast(
                    gths[r][:, k, :], st8s[r][:],
                    rs1 if r == 0 else rs2, lsem, rdests=rd,
                )
                if k == 1:
                    if r == 0:
                        # gate the first sends on the kernel-entry barrier
                        # (prelude AllGather) so no core writes into a peer
                        # that has not entered the kernel yet
                        nc._bir_kernel_barrier_sem_replica_groups.append(
                            set(range(8))
                        )
                        pin_wait(
                            bp,
                            nc._bir_kernel_barrier_sem,
                            nc.bir_kernel_barrier_sem_inc,
                        )
                    else:
                        # round-2 descs reuse the SWDGE ring: wait for
                        # round-1's sends (7 x 16 lane-completions) first
                        pin_wait(bp, lsem, 112)


        def pin_wait(bi, sem, val):
            # Attach a remote-satisfied wait to a data-pinned instruction
            # (standalone waits are dependency-free and the scheduler may
            # hoist them to the front of an engine queue, deadlocking all
            # cores). The value is zeroed during tile scheduling (the
            # single-core sim can't model remote increments) and restored
            # before codegen.
            bi.wait_op(sem, val, "sem-ge")
            rdma_waits.append((bi.ins, sem.num, val))

        st8s = [wgt.tile([P, 8], F32, tag="st8", name=f"st8_{r}") for r in (0, 1)]

        def side_psum():
            # side-work scratch rotates through the energy pool slots
            return pE.tile([P, 2, 512], F32, tag="energy", name="side_ps")

        def keep_warm(ar_idx):
            # PE clock-gate (HAM) re-throttles to 1.2GHz after ~3.4us idle;
            # the allreduce bubble is ~20-35us, so post-BN matmuls would run
            # cold for the first windows. Burn idle-PE cycles on dummy
            # 1-partition matmuls anchored after the stats (via the bf16
            # cast) so they fill the front of the bubble. Bounded to ~10us
            # so they cannot delay the post-BN work if the allreduce is
            # faster than usual.
            wb = ew.tile([P, 1], BF16, tag="warm_b")
            nc.vector.tensor_copy(wb[:], st8s[ar_idx - 1][:, 0:1])
            for i in range(32):
                ps = side_psum()
                nc.tensor.matmul(
                    ps[0:1, 0, :512], wb[:], xp[:, i * 56 : i * 56 + 512],
                    start=True, stop=True,
                )

        def proj_chunk(rr, dst, c):
            q0, qn = QCH[c]
            ps = side_psum()
            for ko in range(NKO):
                nc.tensor.matmul(
                    ps[:, 0, :qn], w_inT_r[:, ko, :], rr[ko][:, q0 : q0 + qn],
                    start=(ko == 0), stop=(ko == NKO - 1),
                )
            nc.vector.tensor_scalar_add(dst[:, q0 : q0 + qn], ps[:, 0, :qn], b_in[:])

        def build_vT_chunk(vT, gwvT_r, mos):
            # vT[mo][m, c] = sum_c' yp[c', mo*P+m] * (gamma*wv^T)[c', c], fp8e4
            for mo in mos:
                pst = side_psum()
                nc.tensor.matmul(
                    pst[:, 0, :P], yp[:, mo * P : (mo + 1) * P], gwvT_r[:],
                    start=True, stop=True,
                )
                nc.vector.tensor_copy(vT[:, mo, :], pst[:, 0, :P])

        def qh_chunk(A_r, src_r, dst, c):
            q0, qn = QCH[c]
            ps = side_psum()
            nc.tensor.matmul(
                ps[:, 0, :qn], A_r[:], src_r[:, q0 : q0 + qn], start=True, stop=True
            )
            nc.vector.tensor_copy(dst[:, q0 : q0 + qn], ps[:, 0, :qn])

        # ---------- helpers ----------
        def zero_pad_border(pad):
            nc.vector.tensor_copy(pad[:, 0, :], zrow[:])
            nc.vector.tensor_copy(pad[:, H + 1, :], zrow[:])
            nc.vector.tensor_copy(pad[:, 1 : H + 1, 0:1], zrow[:, :H, None])
            nc.vector.tensor_copy(pad[:, 1 : H + 1, W + 1 : W + 2], zrow[:, :H, None])

        def attention(key_r, q_r, vT_r, resid_r, pad_tag, side_work=None):
            # key_r: [P, N] stationary side (yp for attn1, ky2 for attn2)
            # q_r:   [P, N] moving side (qh1 for attn1, h2 for attn2)
            pad = act.tile([P, H + 2, W + 2], BF16, tag=pad_tag)
            zero_pad_border(pad)
            pairs = [(qi, j) for qi in range(len(QCH)) for j in range(NPAIR)]

            def emit_energy(qi, j):
                q0, qn = QCH[qi]
                ps_e = pE.tile([P, 2, 512], F32, tag="energy")
                for t in range(2):
                    nc.tensor.matmul(
                        ps_e[:, t, :qn],
                        key_r[:, (2 * j + t) * P : (2 * j + t + 1) * P],
                        q_r[:, q0 : q0 + qn],
                        start=True, stop=True,
                    )
                return ps_e

            def epilogue(qi, ps_o, ps_cs):
                # denominator is pre-broadcast across partitions in PSUM
                q0, qn = QCH[qi]
                rcp = ew.tile([P, 480], F32, tag="recip")
                nc.vector.reciprocal_approx_fast(rcp[:, :qn], ps_cs[:, :qn])
                tmp = ew.tile([P, 480], F32, tag="tmp")
                nc.vector.tensor_tensor(
                    tmp[:, :qn], ps_o[:, :qn], rcp[:, :qn], ALU.mult
                )
                r0, nr = q0 // W, qn // W
                nc.vector.tensor_tensor(
                    pad[:, 1 + r0 : 1 + r0 + nr, 1 : W + 1],
                    tmp[:, :qn].rearrange("p (a b) -> p a b", b=W),
                    resid_r[:, q0 : q0 + qn].rearrange("p (a b) -> p a b", b=W),
                    ALU.add,
                )

            ps_o = ps_cs = None
            pend = emit_energy(*pairs[0])
            for p, (qi, j) in enumerate(pairs):
                q0, qn = QCH[qi]
                if side_work is not None and p in side_work:
                    side_work[p]()
                ps_e = pend
                pend = emit_energy(*pairs[p + 1]) if p + 1 < len(pairs) else None
                if j == 0:
                    ps_o = pAcc.tile([P, 512], F32, tag="acc")
                    ps_cs = pCS.tile([P, 512], F32, tag="cs")
                ee = eeP.tile([P, 2, 512], FP8E5, tag="ee")
                nc.scalar.activation(
                    ee[:, :, :qn], ps_e[:, :, :qn], AF.Exp, bias=exp_bias[:]
                )
                # value + denominator: one DoubleRow matmul each (2 rows/cyc,
                # contraction 256 = the m-chunk pair)
                nc.tensor.matmul(
                    ps_o[:, :qn], vT_r[:, 2 * j : 2 * j + 2, :], ee[:, :, :qn],
                    start=(j == 0), stop=(j == NPAIR - 1), perf_mode=DR,
                )
                nc.tensor.matmul(
                    ps_cs[:, :qn], ones8[:], ee[:, :, :qn],
                    start=(j == 0), stop=(j == NPAIR - 1), perf_mode=DR,
                )
                if j == NPAIR - 1:
                    epilogue(qi, ps_o, ps_cs)
            return pad

        def conv_bn_relu(pad, wT_r, bns, bnb, t_tag, out_tag, ar_idx,
                         overlap_fn=None, defer_relu=False):
            # conv3x3 SAME via 9 shifted-window matmuls; batch-stat allreduce
            t_sb = act.tile([P, N], BF16, tag=t_tag)
            sums = ew.tile([P, len(ROWCH)], F32, tag="sums")
            sqs = ew.tile([P, len(ROWCH)], F32, tag="sqs")
            for ci, (r0, nr) in enumerate(ROWCH):
                qn = nr * W
                ps = pAcc.tile([P, 512], F32, tag="acc")
                t = 0
                for dy in range(3):
                    for dx in range(3):
                        nc.tensor.matmul(
                            ps[:, :qn],
                            wT_r[:, t, :],
                            pad[:, dy + r0 : dy + r0 + nr, dx : dx + W],
                            start=(t == 0),
                            stop=(t == 8),
                        )
                        t += 1
                q0 = r0 * W
                nc.vector.tensor_copy(t_sb[:, q0 : q0 + qn], ps[:, :qn])
                nc.vector.reduce_sum(sums[:, ci : ci + 1], ps[:, :qn], axis=AX.X)
                # square on DVE (not ACT): keeps the scalar engine's spline
                # table set on Exp, avoiding two ~1.3us ACT_TABLE_LOAD swaps
                # per BN that stall the next attention's first EXPs
                scr = ew.tile([P, 480], F32, tag="sq_scr")
                nc.vector.tensor_tensor(
                    scr[:, :qn], ps[:, :qn], t_sb[:, q0 : q0 + qn], ALU.mult
                )
                nc.vector.reduce_sum(sqs[:, ci : ci + 1], scr[:, :qn], axis=AX.X)
            r = ar_idx - 1
            st8 = st8s[r]
            stats = st8[:, 0:2]
            nc.vector.reduce_sum(st8[:, 0:1], sums[:], axis=AX.X)
            nc.vector.reduce_sum(st8[:, 1:2], sqs[:], axis=AX.X)
            nc.vector.tensor_scalar_mul(stats, stats, 1.0 / NSTAT)
            # fold +EPS into the allreduced sumsq (each core adds EPS/8)
            nc.vector.tensor_scalar_add(st8[:, 1:2], st8[:, 1:2], EPS / 8.0)
            red = ew.tile([P, 2], F32, tag="red")
            if rdma:
                gth = gths[r]
                rs = rs1 if r == 0 else rs2
                # own stats into slot 0 (RAW-pins the gathered reduce after
                # the stats computation; the remote wait rides on it)
                nc.vector.tensor_copy(gth[:, 0, :], st8[:])
                # preps MUST be emitted after the stats writes: the prep's
                # source-tensor read is deferred to the trigger, and only a
                # read-dep that exists at emission transfers to the trigger.
                # Emitting preps early lets the scheduler hoist the trigger
                # to kernel start, firing stale stats.
                emit_stat_preps(r)
                # count=None: the framework attaches the Pool engine sem to
                # each prep and gives the trigger a desc-commit wait. The
                # trigger's wait slot MUST stay free for that — an explicit
                # wait here displaces it and the trigger can fire before the
                # last prep commits (observed: one send left un-fired).
                nc.gpsimd.trigger_dma(count=None)
                if overlap_fn is not None:
                    overlap_fn()
                keep_warm(ar_idx)
                ri0 = nc.vector.reduce_sum(red[:, 0:1], gth[:, :, 0], axis=AX.X)
                pin_wait(ri0, rs, 14)
                ri1 = nc.vector.reduce_sum(red[:, 1:2], gth[:, :, 1], axis=AX.X)
                pin_wait(ri1, rs, 14)
            else:
                cc_in = dram.tile([P, 2], F32, tag=f"cc_in{ar_idx}")
                cc_out = dram.tile([P, 2], F32, tag=f"cc_out{ar_idx}")
                nc.sync.dma_start(cc_in[:], stats)
                if sim:
                    nc.sync.dma_start(cc_out[:], cc_in[:])
                else:
                    nc.gpsimd.collective_compute(
                        "AllReduce",
                        ALU.add,
                        replica_groups=[list(range(8))],
                        ins=[cc_in[:].opt()],
                        outs=[cc_out[:].opt()],
                    )
                if overlap_fn is not None:
                    overlap_fn()
                keep_warm(ar_idx)
                nc.sync.dma_start(red[:], cc_out[:])
            mean = red[:, 0:1]
            var = ew.tile([P, 1], F32, tag="var")
            nc.vector.tensor_tensor(var[:], mean, mean, ALU.mult)
            # var+eps = m2e - mean^2, fused: (var * -1 + m2e)
            nc.vector.scalar_tensor_tensor(
                var[:], var[:], -1.0, red[:, 1:2], ALU.mult, ALU.add
            )
            std = ew.tile([P, 1], F32, tag="std")
            nc.scalar.activation(std[:], var[:], AF.Sqrt)
            a_sc = ew.tile([P, 1], F32, tag="a_sc")
            with nc.allow_low_precision(reason="bn rsqrt"):
                nc.vector.reciprocal(a_sc[:], std[:])
            nc.vector.tensor_tensor(a_sc[:], a_sc[:], bns[:], ALU.mult)
            c_bi = ew.tile([P, 1], F32, tag="c_bi")
            nc.vector.tensor_tensor(c_bi[:], mean, a_sc[:], ALU.mult)
            nc.vector.tensor_tensor(c_bi[:], bnb[:], c_bi[:], ALU.subtract)
            h_out = act.tile([P, N], BF16, tag=out_tag)

            def relu_chunk(c):
                q0, qn = QCH[c]
                nc.scalar.activation(
                    h_out[:, q0 : q0 + qn], t_sb[:, q0 : q0 + qn],
                    AF.Relu, bias=c_bi[:], scale=a_sc[:],
                )

            if defer_relu:
                # emit only chunk 0; the caller interleaves chunks 1-4 into
                # the following attention so they don't head-of-line-block
                # that attention's EXPs on the ACT queue
                relu_chunk(0)
                return h_out, relu_chunk
            for c in range(len(QCH)):
                relu_chunk(c)
            return h_out

        # ---------- pipeline ----------
        qh1 = act.tile([P, N], BF16, tag="tagC")
        vT1 = act.tile([P, NMO, P], FP8E4, tag="vT1")
        # prelude: chunk-0 projections so attention 1 can start immediately
        proj_chunk(yr, yp, 0)
        build_vT_chunk(vT1, gwv1T_r, VT_BY_CHUNK[0])
        proj_chunk(xr, xp, 0)
        qh_chunk(A1_r, xp, qh1, 0)

        def mk_side1a(c):
            def f():
                proj_chunk(yr, yp, c)
                build_vT_chunk(vT1, gwv1T_r, VT_BY_CHUNK[c])
            return f

        def mk_side1b(c):
            def f():
                proj_chunk(xr, xp, c)
                qh_chunk(A1_r, xp, qh1, c)
            return f

        side1 = {}
        for c in range(1, 5):
            side1[c - 1] = mk_side1a(c)
            side1[3 + c] = mk_side1b(c)
        h1pad = attention(yp, qh1, vT1, xp, "tagE", side_work=side1)

        # vT2 + ky2 built inside the BN-1 allreduce window
        vT2 = act.tile([P, NMO, P], FP8E4, tag="vT2")
        ky2 = act.tile([P, N], BF16, tag="tagC2")

        def ar1_overlap():
            build_vT_chunk(vT2, gwv2T_r, range(NMO))
            for c in range(len(QCH)):
                qh_chunk(A2T_r, yp, ky2, c)

        h2, relu2_chunk = conv_bn_relu(
            h1pad, w1T_r, bn1s, bn1b, "tagT", "h2", 1,
            overlap_fn=ar1_overlap, defer_relu=True,
        )
        # relu chunk qi must be emitted before qsb qi's first energy matmul,
        # which is emitted one pair ahead (at pair 9*qi - 1)
        side2 = {9 * c - 2: (lambda c=c: relu2_chunk(c)) for c in range(1, 5)}
        h3pad = attention(ky2, h2, vT2, h2, "tagE", side_work=side2)
        r2 = conv_bn_relu(h3pad, w2T_r, bn2s, bn2b, "tagT", "h2", 2)

        if _os.environ.get("KERNEL_DBG", "0") == "1":
            d_dbg = nc.dram_tensor("dbg", [P, 144], F32, kind="ExternalOutput")
            dbg_sb = act.tile([P, 144], F32, tag="dbg_sb")
            nc.vector.tensor_copy(
                dbg_sb[:, 0:64], gths[0][:].rearrange("p a b -> p (a b)")
            )
            nc.vector.tensor_copy(
                dbg_sb[:, 64:128], gths[1][:].rearrange("p a b -> p (a b)")
            )
            nc.vector.tensor_copy(dbg_sb[:, 128:136], st8s[0][:])
            nc.vector.tensor_copy(dbg_sb[:, 136:144], st8s[1][:])
            nc.sync.dma_start(d_dbg[...], dbg_sb[:])

        # ---------- pred head ----------
        out_sb = act.tile([1, N], F32, tag="out_sb")
        for q0, qn in QCH:
            ps = pAcc.tile([P, 512], F32, tag="acc")
            nc.tensor.matmul(
                ps[:, :qn], predT_r[:], r2[:, q0 : q0 + qn], start=True, stop=True
            )
            nc.vector.tensor_scalar_add(
                out_sb[:, q0 : q0 + qn], ps[0:1, :qn], pred_b[:]
            )
            # per-chunk output DMA overlaps the store with later pred chunks
            nc.sync.dma_start(
                d_out[:, q0 : q0 + qn], out_sb[:, q0 : q0 + qn]
            )

        # zero remote-satisfied waits so the single-core scheduling sim (runs
        # at TileContext exit) doesn't deadlock on increments only peers send
        for ins, sem_num, _ in rdma_waits:
            si = ins.sync_info
            for w in si.on_wait:
                if w.id == sem_num:
                    w.wait_value = 0
            ins.sync_info = si

    # scheduling done; restore the real wait values before codegen
    for ins, sem_num, val in rdma_waits:
        si = ins.sync_info
        for w in si.on_wait:
            if w.id == sem_num:
                w.wait_value = val
        ins.sync_info = si
    nc.compile()
    _NC_CACHE[key] = nc
    return nc


def _install_ntff_hook():
    """Register the axon NTFF profiling hook (antenv.axon_hooks is absent
    in this image; libaxon_pjrt.so exports the C ABI — same wiring as
    trn_agent_boot's _ntff_profile_via_ctypes)."""
    import sys as _sys, types, ctypes, contextlib

    if "antenv.axon_hooks" in _sys.modules:
        return
    try:
        lib = ctypes.CDLL("/opt/axon/libaxon_pjrt.so")
        lib.axon_start_nrt_profile.argtypes = [
            ctypes.POINTER(ctypes.c_int64), ctypes.c_size_t,
        ]
        lib.axon_start_nrt_profile.restype = ctypes.c_int64
        lib.axon_stop_nrt_profile.argtypes = [ctypes.c_char_p]
        lib.axon_stop_nrt_profile.restype = ctypes.c_int64
    except (OSError, AttributeError):
        return

    @contextlib.contextmanager
    def _hook(output_dir, device_ids):
        import jax

        jax.devices()
        if device_ids:
            ids = (ctypes.c_int64 * len(device_ids))(*device_ids)
            rc = lib.axon_start_nrt_profile(ids, len(device_ids))
        else:
            rc = lib.axon_start_nrt_profile(None, 0)
        if rc != 0:
            raise RuntimeError(f"axon_start_nrt_profile rc={rc}")
        try:
            yield
        finally:
            n = lib.axon_stop_nrt_profile(str(output_dir).encode())
            if n < 0:
                raise RuntimeError(f"axon_stop_nrt_profile rc={n}")

    mod = types.ModuleType("antenv.axon_hooks")
    mod.get_axon_ntff_profile_hook = lambda: _hook
    mod.set_axon_ntff_profile_hook = lambda h: None
    _sys.modules["antenv.axon_hooks"] = mod
    # artifact upload has no bucket in this container; keep files local
    import concourse.bass_utils as _bu

    _bu.upload_artifacts = lambda d: d


def kernel(**inputs):
    from concourse.bass_utils import run_bass_kernel_spmd
    import ml_dtypes
    import os

    nc = _build()

    f32 = np.float32
    bf16 = ml_dtypes.bfloat16
    x = np.ascontiguousarray(inputs["x"], dtype=f32).reshape(B, CIN, N)
    y = np.ascontiguousarray(inputs["y"], dtype=f32).reshape(B, CIN, N)
    w_in = np.asarray(inputs["w_in"], dtype=f32)
    b_in = np.asarray(inputs["b_in"], dtype=f32).reshape(P, 1)
    ca_wq = np.asarray(inputs["ca_wq"], dtype=f32)
    ca_wk = np.asarray(inputs["ca_wk"], dtype=f32)
    ca_wv = np.asarray(inputs["ca_wv"], dtype=f32)
    g1 = np.asarray(inputs["ca_gamma"], dtype=f32).reshape(-1)[0]
    sa_wq = np.asarray(inputs["sa_wq"], dtype=f32)
    sa_wk = np.asarray(inputs["sa_wk"], dtype=f32)
    sa_wv = np.asarray(inputs["sa_wv"], dtype=f32)
    g2 = np.asarray(inputs["sa_gamma"], dtype=f32).reshape(-1)[0]
    conv1_w = np.asarray(inputs["conv1_w"], dtype=f32)
    conv2_w = np.asarray(inputs["conv2_w"], dtype=f32)
    bn1s = np.asarray(inputs["bn1_s"], dtype=f32).reshape(P, 1)
    bn1b = np.asarray(inputs["bn1_b"], dtype=f32).reshape(P, 1)
    bn2s = np.asarray(inputs["bn2_s"], dtype=f32).reshape(P, 1)
    bn2b = np.asarray(inputs["bn2_b"], dtype=f32).reshape(P, 1)
    pred_w = np.asarray(inputs["pred_w"], dtype=f32)
    pred_b = np.asarray(inputs["pred_b"], dtype=f32).reshape(1, 1)

    # host-side weight prep (small, O(C^2))
    w_inT = np.ascontiguousarray(
        w_in.T.reshape(NKO, P, P).transpose(1, 0, 2)
    ).astype(bf16)  # [cin_p, ko, cout]
    A1 = np.ascontiguousarray(ca_wq.T @ ca_wk).astype(bf16)
    A2T = np.ascontiguousarray((sa_wq.T @ sa_wk).T).astype(bf16)
    gwv1T = np.ascontiguousarray(g1 * ca_wv.T).astype(bf16)
    gwv2T = np.ascontiguousarray(g2 * sa_wv.T).astype(bf16)
    # conv taps: [o, i, 3, 3] -> lhsT per tap [i, o]; layout [i_p, tap, o]
    w1T = np.ascontiguousarray(
        conv1_w.transpose(2, 3, 1, 0).reshape(9, P, P).transpose(1, 0, 2)
    ).astype(bf16)
    w2T = np.ascontiguousarray(
        conv2_w.transpose(2, 3, 1, 0).reshape(9, P, P).transpose(1, 0, 2)
    ).astype(bf16)
    predT = np.zeros((P, P), f32)
    predT[:, 0] = pred_w[0]
    predT = predT.astype(bf16)

    shared = {
        "w_inT": w_inT, "b_in": b_in, "A1": A1, "gwv1T": gwv1T,
        "A2T": A2T, "gwv2T": gwv2T,
        "w1T": w1T, "bn1s": bn1s,
        "bn1b": bn1b, "w2T": w2T, "bn2s": bn2s, "bn2b": bn2b,
        "predT": predT, "pred_b": pred_b,
    }
    in_maps = [
        {
            "x": np.ascontiguousarray(x[i]).astype(bf16),
            "y": np.ascontiguousarray(y[i]).astype(bf16),
            **shared,
        }
        for i in range(B)
    ]

    trace = bool(int(os.environ.get("KERNEL_TRACE", "0")))
    if trace:
        _install_ntff_hook()
    res = run_bass_kernel_spmd(nc, in_maps, core_ids=list(range(B)), trace=trace)
    if trace:
        _NC_CACHE["last_results"] = res
    out = np.stack(
        [res.results[i]["out"].reshape(1, H, W) for i in range(B)]
    ).astype(f32)
    return out
